# revision 15
# baseline (speedup 1.0000x reference)
"""Trainium2 Bass kernel for nn_Convolution (e3nn-style GNN message passing).

Strategy (8 NeuronCores, SPMD, no collectives):
- Sort edges by destination; core c owns destination nodes [6400c, 6400(c+1)).
- Per core: edges are binned into 50 node-blocks (128 nodes each) and padded to
  NG groups of 128 edges per block. Dummy edges gather a zero table row, so
  every tensor-product output term (all bilinear in source features) is 0.
- Gather source features with dma_gather from a 256B-padded bf16 table, one
  512-idx call per window.
- Radial MLP layer 1 on PE with tile_position row-packed K=8 matmuls,
  layer 2 per-group with h as the stationary operand (w lands [edge, 256]).
- All matmuls and DVE product ops in bf16 (PSUM accumulates fp32); fp32 only
  for PSUM accumulators and the final staged output.
- TP products on DVE via broadcast APs; the per-edge contraction over u is
  DEFERRED into the scatter matmul: one-hot(dst) x [512-wide product tile]
  accumulates in PSUM over each block, reduced over u once per block.
"""

import math
import os
import numpy as np
from ml_dtypes import bfloat16 as BF16

_TRACE_SIM = bool(int(os.environ.get('K_TRACE_SIM', '0')))
_NO_GATHER = bool(int(os.environ.get('K_NO_GATHER', '0')))
_NO_TP = bool(int(os.environ.get('K_NO_TP', '0')))
_NO_MM = bool(int(os.environ.get('K_NO_MM', '0')))


import concourse.bass as bass
import concourse.bacc as bacc
import concourse.mybir as mybir
from concourse.tile import TileContext
from concourse.bass_utils import run_bass_kernel_spmd

# ---------------- problem constants (hardcoded per spec) ----------------
N_NODES, N_EDGES, NUM_BASIS, HIDDEN = 50000, 800000, 8, 256
MUL = 8
INV_SQRT3 = float(1.0 / np.sqrt(3.0))
A_SCALAR = float(np.sqrt(1.0 / 128.0))
A_VECTOR = float(np.sqrt(3.0 / 128.0))
SQRT2 = float(np.sqrt(2.0))
DEG_SCALE = float(1.0 / np.sqrt(N_EDGES / N_NODES))

NCORES = 8
P = 128
NODES_PER_CORE = 6400          # 50 blocks of 128; 8*6400 = 51200 >= 50000
NB = 50                        # node blocks per core
# table: rows 1..50000 = nodes 0..49999; row 50001 = zeros (dummy target).
# gather base = row 32768, int16 idx = node - 32767 in [-32767, 17232];
# dummy idx = +17233 (always non-negative so it never hits the trailing-
# negative trim). Each gather's last (trim-order) index is forced >= 0 by an
# in-block edge swap on the host.
TBL_ROWS = 50004
TBL_COLS = 128                 # bf16 -> 256B rows (dma_gather elem granule)
GBASE = 32768
DUMMY_IDX = 50001 - GBASE

_PROG_CACHE = {}


def _dma_gather_thin(g, out_ap, in_ap, idxs_ap, num_idxs, elem_size, elem_step,
                     queue_num):
    """dma_gather variant with elem_size < 256B (table stride stays 256B)."""
    from concourse.bass import exact_div, round_up_to_multiple
    assert idxs_ap.dtype == mybir.dt.int16
    assert in_ap.dtype == out_ap.dtype
    assert in_ap.ap[-1][1] == out_ap.ap[-1][1] == elem_size
    assert out_ap.ap[0][1] * out_ap.ap[1][1] == round_up_to_multiple(num_idxs, 128)
    assert in_ap.ap[0][0] == elem_step
    stride_bytes = elem_step * mybir.dt.size(in_ap.dtype)
    return g.add_instruction(
        mybir.InstDMAGatherAnt(
            name=g.bass.get_next_instruction_name(),
            ins=[*g.lower_ap_dma(in_ap, for_custom_bir_dma=True),
                 g.lower_ap(idxs_ap),
                 g.lower_val_access(g.to_reg(num_idxs))],
            outs=[g.lower_ap(out_ap)],
            transpose=False,
            num_idxs=num_idxs,
            elem_size=elem_size,
            stride_bytes_256=exact_div(stride_bytes, 256),
            gen_mode=0,
            single_packet=True,
            queue_num=queue_num,
            sbuf_tokens_per_rank=0,
            sbuf_free_dim_per_rank=0,
            sbuf_free_dim_pad_per_rank=0,
            sbuf_byte_offset=0,
        )
    )


# ---------------- device program ----------------
def _build_program(schedule):
    GROUPS = sum(schedule)
    WINDOWS = GROUPS // 4            # 4 groups (512 edges) per window
    NSUPER = (WINDOWS + 1) // 2      # one 1024-idx gather per 2 windows
    NJ = (WINDOWS + 3) // 4          # es_w4 column blocks
    ES_CHUNK_J = 3                   # es col-blocks per streamed chunk
    NCHUNK = (NJ + ES_CHUNK_J - 1) // ES_CHUNK_J

    nc = bacc.Bacc(num_devices=NCORES, num_swdge_queues=4)
    f32, i16 = mybir.dt.float32, mybir.dt.int16
    bf16 = mybir.dt.bfloat16

    tbl = nc.dram_tensor("tbl", [TBL_ROWS, TBL_COLS], bf16, kind="ExternalInput")
    idx_g = nc.dram_tensor("idx_g", [P, GROUPS * 8], i16, kind="ExternalInput")
    es4 = nc.dram_tensor("es4", [32, NJ * 512], bf16, kind="ExternalInput")
    sh_t = nc.dram_tensor("sh_t", [P, GROUPS * 4], bf16, kind="ExternalInput")
    oht = nc.dram_tensor("oht", [P, GROUPS * 128], mybir.dt.float8e4,
                         kind="ExternalInput")
    w1t = nc.dram_tensor("w1t", [P, 256], bf16, kind="ExternalInput")
    w2t = nc.dram_tensor("w2t", [P, 512], bf16, kind="ExternalInput")
    nodeout = nc.dram_tensor("nodeout", [NODES_PER_CORE, 32], f32, kind="ExternalOutput")

    gmeta = []                       # group -> (block slot k, gib, ngk)
    for k, ngk in enumerate(schedule):
        for gib in range(ngk):
            gmeta.append((k, gib, ngk))
    assert len(gmeta) == GROUPS

    AX = mybir.AxisListType.X
    ADD = mybir.AluOpType.add
    MUL_ = mybir.AluOpType.mult
    EQ = mybir.AluOpType.is_equal
    RELU = mybir.ActivationFunctionType.Relu

    with TileContext(nc, trace_sim=_TRACE_SIM) as tc:
        with tc.tile_pool(name="const", bufs=1) as cpool, \
             tc.tile_pool(name="stream", bufs=2) as spool, \
             tc.tile_pool(name="gat", bufs=8) as gpool, \
             tc.tile_pool(name="ohp", bufs=4) as ohpool, \
             tc.tile_pool(name="work", bufs=3) as wpool, \
             tc.tile_pool(name="psum", bufs=2, space="PSUM") as pp, \
             tc.tile_pool(name="psum1", bufs=1, space="PSUM") as pp1:

            # constants resident in SBUF
            ig_sb = cpool.tile([P, GROUPS * 8], i16, name="ig")
            nc.sync.dma_start(ig_sb[:], idx_g[:])
            sh_sb = cpool.tile([P, GROUPS, 4], bf16, name="sh")
            nc.sync.dma_start(sh_sb[:], sh_t[:].rearrange("p (g k) -> p g k", k=4))
            w1_sb = cpool.tile([P, 256], bf16, name="w1")
            nc.sync.dma_start(w1_sb[:], w1t[:])
            w2_sb = cpool.tile([P, 2, 256], bf16, name="w2")
            nc.sync.dma_start(w2_sb[:], w2t[:].rearrange("p (h n) -> p h n", h=2))

            state = {"acc": None}

            def emit_scatter(g0, oh4, scat):
                for gg in range(4):
                    g = g0 + gg
                    b, gib, ngk = gmeta[g]
                    if gib == 0:
                        state["acc"] = pp.tile(
                            [P, 512], f32, space="PSUM", tag="acc", name="acc"
                        )
                    acc_ps = state["acc"]
                    nc.tensor.matmul(
                        acc_ps[:],
                        lhsT=oh4[:, gg, :], rhs=scat[:, gg, :],
                        start=(gib == 0), stop=(gib == ngk - 1),
                    )
                    if gib == ngk - 1:
                        stage = wpool.tile([P, 32], f32, tag="stage",
                                           name="stage")
                        nc.vector.tensor_reduce(
                            out=stage[:, 0:8],
                            in_=acc_ps[:, 0:128].rearrange(
                                "p (u w) -> p w u", u=16
                            ),
                            axis=AX, op=ADD,
                        )
                        nc.vector.tensor_reduce(
                            out=stage[:, 8:32],
                            in_=acc_ps[:, 128:512].rearrange(
                                "p (u wi) -> p wi u", u=16
                            ),
                            axis=AX, op=ADD,
                        )
                        nc.sync.dma_start(
                            nodeout[128 * b : 128 * b + 128, :], stage[:]
                        )

            pending = None
            for w in range(WINDOWS):
                c = w % 4
                j = w // 4
                g0 = 4 * w

                # --- stream es chunk (every ES_CHUNK_J col-blocks)
                if j % ES_CHUNK_J == 0:
                    jw = min(ES_CHUNK_J, NJ - j)
                    es_sb = spool.tile([P, ES_CHUNK_J * 512], bf16, tag="es")
                    for cc in range(4):
                        nc.sync.dma_start(
                            es_sb[32 * cc : 32 * cc + 8, : jw * 512],
                            es4[8 * cc : 8 * cc + 8, j * 512 : (j + jw) * 512],
                        )
                jj = j % ES_CHUNK_J

                # --- stream this window's 4 one-hot scatter matrices (fp8),
                # upcast to bf16 on the Act engine (0/1 are exact in fp8)
                oh4f = ohpool.tile([P, 4, P], mybir.dt.float8e4, tag="oh4f")
                nc.sync.dma_start(
                    oh4f[:], oht[:, g0 * 128 : (g0 + 4) * 128].rearrange(
                        "p (g n) -> p g n", g=4
                    )
                )
                oh4 = wpool.tile([P, 4, P], bf16, tag="oh4b")
                nc.scalar.copy(out=oh4[:], in_=oh4f[:])

                # --- gather: one 1024-idx call per 2 windows, cycling queues
                if w % 2 == 0:
                    sgi = w // 2
                    nidx = 1024 if w + 1 < WINDOWS else 512
                    x_sup = gpool.tile([P, 8, 32], bf16, tag="xc")
                    _dma_gather_thin(
                        nc.gpsimd,
                        out_ap=x_sup[:, : nidx // 128, :], in_ap=tbl[GBASE:, :32],
                        idxs_ap=ig_sb[:, sgi * 64 : sgi * 64 + nidx // 16],
                        num_idxs=nidx, elem_size=32, elem_step=TBL_COLS,
                        queue_num=sgi % 4,
                    )
                x_c = x_sup[:, 4 * (w % 2) : 4 * (w % 2) + 4, :]
                # --- MLP1: h[comp, edge] for 512 edges, two 128-comp halves
                h_ps = pp1.tile([P, 2, 512], f32, space="PSUM", tag="hps")
                for half in range(2):
                    nc.tensor.matmul(
                        h_ps[:, half, :],
                        lhsT=w1_sb[32 * c : 32 * c + 8, half * 128 : half * 128 + 128],
                        rhs=es_sb[32 * c : 32 * c + 8, jj * 512 : jj * 512 + 512],
                        start=True, stop=True,
                        tile_position=(32 * c, 0),
                    )
                h_sb = wpool.tile([P, 2, 512], bf16, tag="hsb")
                for half in range(2):
                    nc.scalar.activation(
                        out=h_sb[:, half, :], in_=h_ps[:, half, :], func=RELU
                    )

                # --- MLP2 per group: w[edge, 256] in PSUM
                w_ps = pp.tile([P, 4, 256], f32, space="PSUM", tag="wps")
                for gg in range(4):
                    for half in range(2):
                        nc.tensor.matmul(
                            w_ps[:, gg, :],
                            lhsT=h_sb[:, half, gg * 128 : gg * 128 + 128],
                            rhs=w2_sb[:, half, :],
                            start=(half == 0), stop=(half == 1),
                        )

                # --- TP products (batched over the 4 groups)
                xs = x_c                              # [P, 4, 128] (use cols 0:32)
                shw = sh_sb[:, g0 : g0 + 4, :]        # [P, 4, 4]
                scat = wpool.tile([P, 4, 512], bf16, tag="scat")
                ab16 = wpool.tile([P, 4, 16], bf16, tag="ab16")
                # a[u] = s1[u] * s2
                nc.vector.tensor_tensor(
                    out=ab16[:, :, 0:8], in0=xs[:, :, 0:8],
                    in1=shw[:, :, 0:1].to_broadcast([P, 4, 8]), op=MUL_,
                )
                # b[u] = sum_i v1[u,i] * v2[i]
                pb = wpool.tile([P, 4, 8, 3], bf16, tag="pb")
                nc.vector.tensor_tensor(
                    out=pb[:],
                    in0=xs[:, :, 8:32].rearrange("p g (u i) -> p g u i", u=8),
                    in1=shw[:, :, 1:4].unsqueeze(2).to_broadcast([P, 4, 8, 3]),
                    op=MUL_,
                )
                with nc.allow_low_precision(reason="3-elem dot, tol 2e-2"):
                    nc.vector.tensor_reduce(
                        out=ab16[:, :, 8:16], in_=pb[:], axis=AX, op=ADD
                    )
                # ps = ab16[u'] * w01[u', w_]  -> scat[:, :, 0:128]
                nc.vector.tensor_tensor(
                    out=scat[:, :, 0:128].rearrange("p g (u w) -> p g u w", u=16),
                    in0=ab16[:].unsqueeze(3).to_broadcast([P, 4, 16, 8]),
                    in1=w_ps[:, :, 0:128].rearrange("p g (u w) -> p g u w", u=16),
                    op=MUL_,
                )
                # z23[t,u,i]: t=0 -> s1[u]*v2[i], t=1 -> v1[u,i]*s2
                z23 = wpool.tile([P, 4, 2, 24], bf16, tag="z23")
                nc.vector.tensor_tensor(
                    out=z23[:, :, 0, :].rearrange("p g (u i) -> p g u i", u=8),
                    in0=xs[:, :, 0:8].unsqueeze(3).to_broadcast([P, 4, 8, 3]),
                    in1=shw[:, :, 1:4].unsqueeze(2).to_broadcast([P, 4, 8, 3]),
                    op=MUL_,
                )
                nc.vector.tensor_tensor(
                    out=z23[:, :, 1, :], in0=xs[:, :, 8:32],
                    in1=shw[:, :, 0:1].to_broadcast([P, 4, 24]), op=MUL_,
                )
                # scat cols 128:512 ((t,u,w,i)) = z23[t,u,i] * w23[t,u,w]
                nc.vector.tensor_tensor(
                    out=scat[:, :, 128:512].rearrange(
                        "p g (t u w i) -> p g t u w i", t=2, u=8, w=8
                    ),
                    in0=z23[:].rearrange("p g t (u i) -> p g t u i", u=8)
                        .unsqueeze(4).to_broadcast([P, 4, 2, 8, 8, 3]),
                    in1=w_ps[:, :, 128:256]
                        .rearrange("p g (t u w) -> p g t u w", t=2, u=8, w=8)
                        .unsqueeze(5).to_broadcast([P, 4, 2, 8, 8, 3]),
                    op=MUL_,
                )

                # --- scatter matmuls are deferred one window so the PE
                # queue never blocks on this window's DVE products: emit the
                # PREVIOUS window's scatters now (their scat tile is ready).
                if pending is not None:
                    emit_scatter(*pending)
                pending = (g0, oh4, scat)
            emit_scatter(*pending)
    nc.compile()
    return nc


# ---------------- host-side prep ----------------
def _prep(node_features, edge_src, edge_dst, edge_sh, edge_scalars, fc_w1, fc_w2,
          schedule):
    GROUPS = sum(schedule)
    EPAD = GROUPS * 128
    Gq = np.zeros(NB + 1, np.int64)
    Gq[1:] = np.cumsum(schedule)
    Gedge = Gq * 128                  # edge-slot start of block slot k
    WINDOWS = GROUPS // 4
    NJ = (WINDOWS + 3) // 4
    # gather call boundaries: 1024-idx supers, possibly a 512 tail
    call_ends = []
    wpos = 0
    while wpos < WINDOWS:
        nidx = 1024 if wpos + 1 < WINDOWS else 512
        call_ends.append(wpos * 512 + nidx)
        wpos += 2

    # fold all scalar coefficients into the weights
    w1s = (fc_w1 * (1.0 / math.sqrt(NUM_BASIS))).astype(np.float32)     # [8, 256]
    w2 = (fc_w2 * (SQRT2 / math.sqrt(HIDDEN))).astype(np.float64)       # [256, 256]
    w2 = w2.reshape(HIDDEN, 4, MUL, MUL)
    coef = np.array(
        [A_SCALAR, A_SCALAR * INV_SQRT3, A_VECTOR * INV_SQRT3, A_VECTOR * INV_SQRT3]
    ) * DEG_SCALE
    w2 = w2 * coef[None, :, None, None]
    # device col order: [w01 (u'16, w8) | w2 (u8, w8) | w3 (u8, w8)]
    w2dev = np.concatenate(
        [
            w2[:, 0].reshape(HIDDEN, 64),
            w2[:, 1].reshape(HIDDEN, 64),
            w2[:, 2].reshape(HIDDEN, 64),
            w2[:, 3].reshape(HIDDEN, 64),
        ],
        axis=1,
    ).astype(np.float32)                                                # [256, 256]

    w1t = np.zeros((P, 256), np.float32)
    for c in range(4):
        w1t[32 * c : 32 * c + 8] = w1s
    w2t = np.zeros((P, 512), np.float32)
    w2t[:, 0:256] = w2dev[0:128]
    w2t[:, 256:512] = w2dev[128:256]
    tbl = np.zeros((TBL_ROWS, TBL_COLS), BF16)
    tbl[1 : N_NODES + 1, 0:32] = node_features

    src_all = np.asarray(edge_src).astype(np.int64)
    dst_all = np.asarray(edge_dst).astype(np.int64)
    es_all = np.asarray(edge_scalars).astype(np.float32)
    sh_all = np.asarray(edge_sh).astype(np.float32)
    core_of = dst_all // NODES_PER_CORE

    in_maps = []
    orders = []
    for cid in range(NCORES):
        sel = np.nonzero(core_of == cid)[0]
        d = dst_all[sel]
        order = np.argsort(d, kind="stable")
        sel = sel[order]
        d = d[order]
        blk = (d - NODES_PER_CORE * cid) >> 7
        cnt = np.bincount(blk, minlength=NB)
        border = np.argsort(-cnt, kind="stable")   # block id for slot k
        orders.append(border)
        kpos = np.empty(NB, np.int64)
        kpos[border] = np.arange(NB)
        assert (cnt[border] <= np.asarray(schedule) * 128).all(), cid
        start = np.zeros(NB, np.int64)
        start[1:] = np.cumsum(cnt)[:-1]
        rank = np.arange(len(sel)) - start[blk]
        slot = Gedge[kpos[blk]] + rank

        srcv = np.full(EPAD, -1, np.int64)
        srcv[slot] = src_all[sel]
        shv = np.zeros((EPAD, 4), np.float32)
        shv[slot] = sh_all[sel]
        esv = np.zeros((EPAD, 8), np.float32)
        esv[slot] = es_all[sel]
        dlv = np.zeros(EPAD, np.float32)
        dlv[slot] = (d - NODES_PER_CORE * cid - 128 * blk).astype(np.float32)

        # --- gather indices: idx = node - (GBASE - 1 - 1)... row = node+1,
        # idx = row - GBASE = node + 1 - GBASE; dummy -> DUMMY_IDX (>= 0)
        idxv = np.where(srcv >= 0, srcv + 1 - GBASE, DUMMY_IDX).astype(np.int64)
        # force the trim-order-last index of each 512-idx gather call to be
        # >= 0 by swapping that edge with a non-negative-idx edge of the SAME
        # node block (any within-block permutation is valid).
        ends = set(call_ends)
        for jl in call_ends:
            jl = jl - 1
            if idxv[jl] >= 0:
                continue
            k0 = int(np.searchsorted(Gedge, jl, side="right")) - 1
            lo, hi = int(Gedge[k0]), int(Gedge[k0 + 1])
            cand = np.nonzero(idxv[lo:hi] >= 0)[0]
            # exclude other calls' final slots
            cand = [lo + q for q in cand if (lo + q + 1) not in ends]
            assert cand, "no swap candidate in block"
            q = cand[0]
            for arr in (idxv, srcv, dlv):
                arr[jl], arr[q] = arr[q], arr[jl]
            for arr in (shv, esv):
                tmpq = arr[q].copy()
                arr[q] = arr[jl]
                arr[jl] = tmpq
        idx_g = np.tile(
            idxv.reshape(-1, 16).T.astype(np.int16), (8, 1)
        )  # wrap is uniform: IDXW*NSUPER cols total

        # es4: window w at rows 32*(w%4)+b, cols [ (w//4)*512, +512 )
        es4 = np.zeros((32, NJ * 512), np.float32)
        esw = esv.reshape(WINDOWS, 512, 8)
        for c in range(4):
            wsel = np.arange(c, WINDOWS, 4)       # these windows use strip c
            nw = len(wsel)                        # w//4 == index within wsel
            es4[8 * c : 8 * c + 8, : nw * 512] = (
                esw[wsel].transpose(2, 0, 1).reshape(8, nw * 512)
            )

        sh_t = shv.reshape(GROUPS, P, 4).transpose(1, 0, 2).reshape(P, GROUPS * 4)
        dl_mat = dlv.reshape(GROUPS, P).astype(np.int32)
        oht = (dl_mat[:, :, None] == np.arange(P, dtype=np.int32)[None, None, :])
        oht = np.ascontiguousarray(
            oht.transpose(1, 0, 2).reshape(P, GROUPS * 128)
        ).astype(mybir.dt.np(mybir.dt.float8e4))

        in_maps.append(
            dict(
                tbl=tbl, idx_g=np.ascontiguousarray(idx_g),
                es4=np.ascontiguousarray(es4).astype(BF16),
                sh_t=np.ascontiguousarray(sh_t).astype(BF16),
                oht=oht,
                w1t=w1t.astype(BF16), w2t=w2t.astype(BF16),
            )
        )
    return in_maps, orders


def _compute_schedule(edge_dst):
    dst_all = np.asarray(edge_dst).astype(np.int64)
    cnt = np.zeros((NCORES, NB), np.int64)
    for c in range(NCORES):
        dl = dst_all[(dst_all // NODES_PER_CORE) == c] - c * NODES_PER_CORE
        cnt[c] = np.bincount(dl >> 7, minlength=NB)
    scnt = -np.sort(-cnt, axis=1)                  # each core's blocks desc
    ng_k = np.maximum(np.ceil(scnt.max(axis=0) / 128.0).astype(np.int64), 1)
    while int(ng_k.sum()) % 4:
        ng_k[-1] += 1
    return tuple(int(x) for x in ng_k)


def kernel(node_features, edge_src, edge_dst, edge_sh, edge_scalars, fc_w1, fc_w2):
    node_features = np.asarray(node_features, dtype=np.float32)
    edge_sh = np.asarray(edge_sh, dtype=np.float32)
    edge_scalars = np.asarray(edge_scalars, dtype=np.float32)
    fc_w1 = np.asarray(fc_w1, dtype=np.float32)
    fc_w2 = np.asarray(fc_w2, dtype=np.float32)

    schedule = _compute_schedule(edge_dst)
    if schedule not in _PROG_CACHE:
        _PROG_CACHE[schedule] = _build_program(schedule)
    nc = _PROG_CACHE[schedule]

    in_maps, orders = _prep(
        node_features, edge_src, edge_dst, edge_sh, edge_scalars, fc_w1, fc_w2,
        schedule,
    )
    res = run_bass_kernel_spmd(nc, in_maps, core_ids=list(range(NCORES)))
    outs = []
    for c in range(NCORES):
        no = res.results[c]["nodeout"].reshape(NB, 128, 32)
        oc = np.empty_like(no)
        oc[orders[c]] = no
        outs.append(oc.reshape(NODES_PER_CORE, 32))
    out = np.concatenate(outs, axis=0)
    return out[:N_NODES].astype(np.float32)


# revision 17
# speedup vs baseline: 1.0037x; 1.0037x over previous
"""Trainium2 Bass kernel for nn_Convolution (e3nn-style GNN message passing).

Strategy (8 NeuronCores, SPMD, no collectives):
- Sort edges by destination; core c owns destination nodes [6400c, 6400(c+1)).
- Per core: edges are binned into 50 node-blocks (128 nodes each) and padded to
  NG groups of 128 edges per block. Dummy edges gather a zero table row, so
  every tensor-product output term (all bilinear in source features) is 0.
- Gather source features with dma_gather from a 256B-padded bf16 table, one
  512-idx call per window.
- Radial MLP layer 1 on PE with tile_position row-packed K=8 matmuls,
  layer 2 per-group with h as the stationary operand (w lands [edge, 256]).
- All matmuls and DVE product ops in bf16 (PSUM accumulates fp32); fp32 only
  for PSUM accumulators and the final staged output.
- TP products on DVE via broadcast APs; the per-edge contraction over u is
  DEFERRED into the scatter matmul: one-hot(dst) x [512-wide product tile]
  accumulates in PSUM over each block, reduced over u once per block.
"""

import math
import os
import numpy as np
from ml_dtypes import bfloat16 as BF16

_TRACE_SIM = bool(int(os.environ.get('K_TRACE_SIM', '0')))
_NO_GATHER = bool(int(os.environ.get('K_NO_GATHER', '0')))
_NO_TP = bool(int(os.environ.get('K_NO_TP', '0')))
_NO_MM = bool(int(os.environ.get('K_NO_MM', '0')))


import concourse.bass as bass
import concourse.bacc as bacc
import concourse.mybir as mybir
from concourse.tile import TileContext
from concourse.bass_utils import run_bass_kernel_spmd

# ---------------- problem constants (hardcoded per spec) ----------------
N_NODES, N_EDGES, NUM_BASIS, HIDDEN = 50000, 800000, 8, 256
MUL = 8
INV_SQRT3 = float(1.0 / np.sqrt(3.0))
A_SCALAR = float(np.sqrt(1.0 / 128.0))
A_VECTOR = float(np.sqrt(3.0 / 128.0))
SQRT2 = float(np.sqrt(2.0))
DEG_SCALE = float(1.0 / np.sqrt(N_EDGES / N_NODES))

NCORES = 8
P = 128
NODES_PER_CORE = 6400          # 50 blocks of 128; 8*6400 = 51200 >= 50000
NB = 50                        # node blocks per core
# table: rows 1..50000 = nodes 0..49999; row 50001 = zeros (dummy target).
# gather base = row 32768, int16 idx = node - 32767 in [-32767, 17232];
# dummy idx = +17233 (always non-negative so it never hits the trailing-
# negative trim). Each gather's last (trim-order) index is forced >= 0 by an
# in-block edge swap on the host.
TBL_ROWS = 50004
TBL_COLS = 128                 # bf16 -> 256B rows (dma_gather elem granule)
GBASE = 32768
DUMMY_IDX = 50001 - GBASE

_PROG_CACHE = {}


def _dma_gather_thin(g, out_ap, in_ap, idxs_ap, num_idxs, elem_size, elem_step,
                     queue_num):
    """dma_gather variant with elem_size < 256B (table stride stays 256B)."""
    from concourse.bass import exact_div, round_up_to_multiple
    assert idxs_ap.dtype == mybir.dt.int16
    assert in_ap.dtype == out_ap.dtype
    assert in_ap.ap[-1][1] == out_ap.ap[-1][1] == elem_size
    assert out_ap.ap[0][1] * out_ap.ap[1][1] == round_up_to_multiple(num_idxs, 128)
    assert in_ap.ap[0][0] == elem_step
    stride_bytes = elem_step * mybir.dt.size(in_ap.dtype)
    return g.add_instruction(
        mybir.InstDMAGatherAnt(
            name=g.bass.get_next_instruction_name(),
            ins=[*g.lower_ap_dma(in_ap, for_custom_bir_dma=True),
                 g.lower_ap(idxs_ap),
                 g.lower_val_access(g.to_reg(num_idxs))],
            outs=[g.lower_ap(out_ap)],
            transpose=False,
            num_idxs=num_idxs,
            elem_size=elem_size,
            stride_bytes_256=exact_div(stride_bytes, 256),
            gen_mode=0,
            single_packet=True,
            queue_num=queue_num,
            sbuf_tokens_per_rank=0,
            sbuf_free_dim_per_rank=0,
            sbuf_free_dim_pad_per_rank=0,
            sbuf_byte_offset=0,
        )
    )


# ---------------- device program ----------------
def _build_program(schedule):
    GROUPS = sum(schedule)
    WINDOWS = GROUPS // 4            # 4 groups (512 edges) per window
    NSUPER = (WINDOWS + 1) // 2      # one 1024-idx gather per 2 windows
    NJ = (WINDOWS + 3) // 4          # es_w4 column blocks
    ES_CHUNK_J = 3                   # es col-blocks per streamed chunk
    NCHUNK = (NJ + ES_CHUNK_J - 1) // ES_CHUNK_J

    nc = bacc.Bacc(num_devices=NCORES, num_swdge_queues=4)
    f32, i16 = mybir.dt.float32, mybir.dt.int16
    bf16 = mybir.dt.bfloat16

    tbl = nc.dram_tensor("tbl", [TBL_ROWS, TBL_COLS], bf16, kind="ExternalInput")
    idx_g = nc.dram_tensor("idx_g", [P, GROUPS * 8], i16, kind="ExternalInput")
    es4 = nc.dram_tensor("es4", [32, NJ * 512], bf16, kind="ExternalInput")
    sh_t = nc.dram_tensor("sh_t", [P, GROUPS * 4], bf16, kind="ExternalInput")
    oht = nc.dram_tensor("oht", [P, GROUPS * 128], mybir.dt.float8e4,
                         kind="ExternalInput")
    w1t = nc.dram_tensor("w1t", [P, 256], bf16, kind="ExternalInput")
    w2t = nc.dram_tensor("w2t", [P, 512], bf16, kind="ExternalInput")
    nodeout = nc.dram_tensor("nodeout", [NODES_PER_CORE, 32], f32, kind="ExternalOutput")

    gmeta = []                       # group -> (block slot k, gib, ngk)
    for k, ngk in enumerate(schedule):
        for gib in range(ngk):
            gmeta.append((k, gib, ngk))
    assert len(gmeta) == GROUPS

    AX = mybir.AxisListType.X
    ADD = mybir.AluOpType.add
    MUL_ = mybir.AluOpType.mult
    EQ = mybir.AluOpType.is_equal
    RELU = mybir.ActivationFunctionType.Relu

    with TileContext(nc, trace_sim=_TRACE_SIM) as tc:
        with tc.tile_pool(name="const", bufs=1) as cpool, \
             tc.tile_pool(name="stream", bufs=2) as spool, \
             tc.tile_pool(name="gat", bufs=8) as gpool, \
             tc.tile_pool(name="ohp", bufs=4) as ohpool, \
             tc.tile_pool(name="work", bufs=3) as wpool, \
             tc.tile_pool(name="psum", bufs=2, space="PSUM") as pp, \
             tc.tile_pool(name="psum1", bufs=1, space="PSUM") as pp1:

            # constants resident in SBUF
            ig_sb = cpool.tile([P, GROUPS * 8], i16, name="ig")
            nc.sync.dma_start(ig_sb[:], idx_g[:])
            sh_sb = cpool.tile([P, GROUPS, 4], bf16, name="sh")
            nc.sync.dma_start(sh_sb[:], sh_t[:].rearrange("p (g k) -> p g k", k=4))
            w1_sb = cpool.tile([P, 256], bf16, name="w1")
            nc.sync.dma_start(w1_sb[:], w1t[:])
            w2_sb = cpool.tile([P, 2, 256], bf16, name="w2")
            nc.sync.dma_start(w2_sb[:], w2t[:].rearrange("p (h n) -> p h n", h=2))

            state = {"acc": None}

            def emit_scatter(g0, oh4, scat):
                for gg in range(4):
                    g = g0 + gg
                    b, gib, ngk = gmeta[g]
                    if gib == 0:
                        state["acc"] = pp.tile(
                            [P, 512], f32, space="PSUM", tag="acc", name="acc"
                        )
                    acc_ps = state["acc"]
                    nc.tensor.matmul(
                        acc_ps[:],
                        lhsT=oh4[:, gg, :], rhs=scat[:, gg, :],
                        start=(gib == 0), stop=(gib == ngk - 1),
                    )
                    if gib == ngk - 1:
                        stage = wpool.tile([P, 32], f32, tag="stage",
                                           name="stage")
                        nc.vector.tensor_reduce(
                            out=stage[:, 0:8],
                            in_=acc_ps[:, 0:128].rearrange(
                                "p (u w) -> p w u", u=16
                            ),
                            axis=AX, op=ADD,
                        )
                        nc.vector.tensor_reduce(
                            out=stage[:, 8:32],
                            in_=acc_ps[:, 128:512].rearrange(
                                "p (u wi) -> p wi u", u=16
                            ),
                            axis=AX, op=ADD,
                        )
                        nc.sync.dma_start(
                            nodeout[128 * b : 128 * b + 128, :], stage[:]
                        )

            pending = None
            for w in range(WINDOWS):
                c = w % 4
                j = w // 4
                g0 = 4 * w

                # --- stream es chunk (every ES_CHUNK_J col-blocks)
                if j % ES_CHUNK_J == 0:
                    jw = min(ES_CHUNK_J, NJ - j)
                    es_sb = spool.tile([P, ES_CHUNK_J * 512], bf16, tag="es")
                    for cc in range(4):
                        nc.sync.dma_start(
                            es_sb[32 * cc : 32 * cc + 8, : jw * 512],
                            es4[8 * cc : 8 * cc + 8, j * 512 : (j + jw) * 512],
                        )
                jj = j % ES_CHUNK_J

                # --- stream this window's 4 one-hot scatter matrices (fp8),
                # upcast to bf16 on the Act engine (0/1 are exact in fp8)
                oh4f = ohpool.tile([P, 4, P], mybir.dt.float8e4, tag="oh4f")
                nc.sync.dma_start(
                    oh4f[:], oht[:, g0 * 128 : (g0 + 4) * 128].rearrange(
                        "p (g n) -> p g n", g=4
                    )
                )
                oh4 = wpool.tile([P, 4, P], bf16, tag="oh4b")
                nc.scalar.copy(out=oh4[:], in_=oh4f[:])

                # --- gather: one 1024-idx call per 2 windows, cycling queues
                if w % 2 == 0:
                    sgi = w // 2
                    nidx = 1024 if w + 1 < WINDOWS else 512
                    x_sup = gpool.tile([P, 8, 32], bf16, tag="xc")
                    _dma_gather_thin(
                        nc.gpsimd,
                        out_ap=x_sup[:, : nidx // 128, :], in_ap=tbl[GBASE:, :32],
                        idxs_ap=ig_sb[:, sgi * 64 : sgi * 64 + nidx // 16],
                        num_idxs=nidx, elem_size=32, elem_step=TBL_COLS,
                        queue_num=sgi % 4,
                    )
                x_c = x_sup[:, 4 * (w % 2) : 4 * (w % 2) + 4, :]
                # --- MLP1: h[comp, edge] for 512 edges, two 128-comp halves
                h_ps = pp1.tile([P, 2, 512], f32, space="PSUM", tag="hps")
                for half in range(2):
                    nc.tensor.matmul(
                        h_ps[:, half, :],
                        lhsT=w1_sb[32 * c : 32 * c + 8, half * 128 : half * 128 + 128],
                        rhs=es_sb[32 * c : 32 * c + 8, jj * 512 : jj * 512 + 512],
                        start=True, stop=True,
                        tile_position=(32 * c, 0),
                    )
                h_sb = wpool.tile([P, 2, 512], bf16, tag="hsb")
                for half in range(2):
                    nc.scalar.activation(
                        out=h_sb[:, half, :], in_=h_ps[:, half, :], func=RELU
                    )

                # --- MLP2 per group: w[edge, 256] in PSUM
                w_ps = pp.tile([P, 4, 256], f32, space="PSUM", tag="wps")
                for gg in range(4):
                    for half in range(2):
                        nc.tensor.matmul(
                            w_ps[:, gg, :],
                            lhsT=h_sb[:, half, gg * 128 : gg * 128 + 128],
                            rhs=w2_sb[:, half, :],
                            start=(half == 0), stop=(half == 1),
                        )

                # --- TP products (batched over the 4 groups)
                xs = x_c                              # [P, 4, 128] (use cols 0:32)
                shw = sh_sb[:, g0 : g0 + 4, :]        # [P, 4, 4]
                scat = wpool.tile([P, 4, 512], bf16, tag="scat")
                ab16 = wpool.tile([P, 4, 16], bf16, tag="ab16")
                # a[u] = s1[u] * s2
                nc.vector.tensor_tensor(
                    out=ab16[:, :, 0:8], in0=xs[:, :, 0:8],
                    in1=shw[:, :, 0:1].to_broadcast([P, 4, 8]), op=MUL_,
                )
                # b[u] = sum_i v1[u,i] * v2[i]
                pb = wpool.tile([P, 4, 8, 3], bf16, tag="pb")
                nc.vector.tensor_tensor(
                    out=pb[:],
                    in0=xs[:, :, 8:32].rearrange("p g (u i) -> p g u i", u=8),
                    in1=shw[:, :, 1:4].unsqueeze(2).to_broadcast([P, 4, 8, 3]),
                    op=MUL_,
                )
                with nc.allow_low_precision(reason="3-elem dot, tol 2e-2"):
                    nc.vector.tensor_reduce(
                        out=ab16[:, :, 8:16], in_=pb[:], axis=AX, op=ADD
                    )
                # ps = ab16[u'] * w01[u', w_]  -> scat[:, :, 0:128]
                nc.vector.tensor_tensor(
                    out=scat[:, :, 0:128].rearrange("p g (u w) -> p g u w", u=16),
                    in0=ab16[:].unsqueeze(3).to_broadcast([P, 4, 16, 8]),
                    in1=w_ps[:, :, 0:128].rearrange("p g (u w) -> p g u w", u=16),
                    op=MUL_,
                )
                # z23[t,u,i]: t=0 -> s1[u]*v2[i], t=1 -> v1[u,i]*s2
                z23 = wpool.tile([P, 4, 2, 24], bf16, tag="z23")
                nc.vector.tensor_tensor(
                    out=z23[:, :, 0, :].rearrange("p g (u i) -> p g u i", u=8),
                    in0=xs[:, :, 0:8].unsqueeze(3).to_broadcast([P, 4, 8, 3]),
                    in1=shw[:, :, 1:4].unsqueeze(2).to_broadcast([P, 4, 8, 3]),
                    op=MUL_,
                )
                nc.vector.tensor_tensor(
                    out=z23[:, :, 1, :], in0=xs[:, :, 8:32],
                    in1=shw[:, :, 0:1].to_broadcast([P, 4, 24]), op=MUL_,
                )
                # scat cols 128:512 ((t,u,w,i)) = z23[t,u,i] * w23[t,u,w]
                nc.vector.tensor_tensor(
                    out=scat[:, :, 128:512].rearrange(
                        "p g (t u w i) -> p g t u w i", t=2, u=8, w=8
                    ),
                    in0=z23[:].rearrange("p g t (u i) -> p g t u i", u=8)
                        .unsqueeze(4).to_broadcast([P, 4, 2, 8, 8, 3]),
                    in1=w_ps[:, :, 128:256]
                        .rearrange("p g (t u w) -> p g t u w", t=2, u=8, w=8)
                        .unsqueeze(5).to_broadcast([P, 4, 2, 8, 8, 3]),
                    op=MUL_,
                )

                # --- scatter matmuls are deferred one window so the PE
                # queue never blocks on this window's DVE products: emit the
                # PREVIOUS window's scatters now (their scat tile is ready).
                if pending is not None:
                    emit_scatter(*pending)
                pending = (g0, oh4, scat)
            emit_scatter(*pending)
    nc.compile()
    return nc


# ---------------- host-side prep ----------------
def _prep(node_features, edge_src, edge_dst, edge_sh, edge_scalars, fc_w1, fc_w2,
          schedule):
    GROUPS = sum(schedule)
    EPAD = GROUPS * 128
    Gq = np.zeros(NB + 1, np.int64)
    Gq[1:] = np.cumsum(schedule)
    Gedge = Gq * 128                  # edge-slot start of block slot k
    WINDOWS = GROUPS // 4
    NJ = (WINDOWS + 3) // 4
    # gather call boundaries: 1024-idx supers, possibly a 512 tail
    call_ends = []
    wpos = 0
    while wpos < WINDOWS:
        nidx = 1024 if wpos + 1 < WINDOWS else 512
        call_ends.append(wpos * 512 + nidx)
        wpos += 2

    # fold all scalar coefficients into the weights
    w1s = (fc_w1 * (1.0 / math.sqrt(NUM_BASIS))).astype(np.float32)     # [8, 256]
    w2 = (fc_w2 * (SQRT2 / math.sqrt(HIDDEN))).astype(np.float64)       # [256, 256]
    w2 = w2.reshape(HIDDEN, 4, MUL, MUL)
    coef = np.array(
        [A_SCALAR, A_SCALAR * INV_SQRT3, A_VECTOR * INV_SQRT3, A_VECTOR * INV_SQRT3]
    ) * DEG_SCALE
    w2 = w2 * coef[None, :, None, None]
    # device col order: [w01 (u'16, w8) | w2 (u8, w8) | w3 (u8, w8)]
    w2dev = np.concatenate(
        [
            w2[:, 0].reshape(HIDDEN, 64),
            w2[:, 1].reshape(HIDDEN, 64),
            w2[:, 2].reshape(HIDDEN, 64),
            w2[:, 3].reshape(HIDDEN, 64),
        ],
        axis=1,
    ).astype(np.float32)                                                # [256, 256]

    w1t = np.zeros((P, 256), np.float32)
    for c in range(4):
        w1t[32 * c : 32 * c + 8] = w1s
    w2t = np.zeros((P, 512), np.float32)
    w2t[:, 0:256] = w2dev[0:128]
    w2t[:, 256:512] = w2dev[128:256]
    tbl = np.zeros((TBL_ROWS, TBL_COLS), BF16)
    tbl[1 : N_NODES + 1, 0:32] = node_features

    src_all = np.asarray(edge_src).astype(np.int64)
    dst_all = np.asarray(edge_dst).astype(np.int64)
    es_all = np.asarray(edge_scalars).astype(np.float32)
    sh_all = np.asarray(edge_sh).astype(np.float32)
    core_of = dst_all // NODES_PER_CORE

    in_maps = []
    orders = []
    for cid in range(NCORES):
        sel = np.nonzero(core_of == cid)[0]
        d = dst_all[sel]
        order = np.argsort(d, kind="stable")
        sel = sel[order]
        d = d[order]
        blk = (d - NODES_PER_CORE * cid) >> 7
        cnt = np.bincount(blk, minlength=NB)
        border = np.argsort(-cnt, kind="stable")   # block id for slot k
        orders.append(border)
        kpos = np.empty(NB, np.int64)
        kpos[border] = np.arange(NB)
        assert (cnt[border] <= np.asarray(schedule) * 128).all(), cid
        start = np.zeros(NB, np.int64)
        start[1:] = np.cumsum(cnt)[:-1]
        rank = np.arange(len(sel)) - start[blk]
        slot = Gedge[kpos[blk]] + rank

        srcv = np.full(EPAD, -1, np.int64)
        srcv[slot] = src_all[sel]
        shv = np.zeros((EPAD, 4), np.float32)
        shv[slot] = sh_all[sel]
        esv = np.zeros((EPAD, 8), np.float32)
        esv[slot] = es_all[sel]
        dlv = np.zeros(EPAD, np.float32)
        dlv[slot] = (d - NODES_PER_CORE * cid - 128 * blk).astype(np.float32)

        # --- gather indices: idx = node - (GBASE - 1 - 1)... row = node+1,
        # idx = row - GBASE = node + 1 - GBASE; dummy -> DUMMY_IDX (>= 0)
        idxv = np.where(srcv >= 0, srcv + 1 - GBASE, DUMMY_IDX).astype(np.int64)
        # force the trim-order-last index of each 512-idx gather call to be
        # >= 0 by swapping that edge with a non-negative-idx edge of the SAME
        # node block (any within-block permutation is valid).
        ends = set(call_ends)
        for jl in call_ends:
            jl = jl - 1
            if idxv[jl] >= 0:
                continue
            k0 = int(np.searchsorted(Gedge, jl, side="right")) - 1
            lo, hi = int(Gedge[k0]), int(Gedge[k0 + 1])
            cand = np.nonzero(idxv[lo:hi] >= 0)[0]
            # exclude other calls' final slots
            cand = [lo + q for q in cand if (lo + q + 1) not in ends]
            assert cand, "no swap candidate in block"
            q = cand[0]
            for arr in (idxv, srcv, dlv):
                arr[jl], arr[q] = arr[q], arr[jl]
            for arr in (shv, esv):
                tmpq = arr[q].copy()
                arr[q] = arr[jl]
                arr[jl] = tmpq
        idx_g = np.tile(
            idxv.reshape(-1, 16).T.astype(np.int16), (8, 1)
        )  # wrap is uniform: IDXW*NSUPER cols total

        # es4: window w at rows 32*(w%4)+b, cols [ (w//4)*512, +512 )
        es4 = np.zeros((32, NJ * 512), np.float32)
        esw = esv.reshape(WINDOWS, 512, 8)
        for c in range(4):
            wsel = np.arange(c, WINDOWS, 4)       # these windows use strip c
            nw = len(wsel)                        # w//4 == index within wsel
            es4[8 * c : 8 * c + 8, : nw * 512] = (
                esw[wsel].transpose(2, 0, 1).reshape(8, nw * 512)
            )

        sh_t = shv.reshape(GROUPS, P, 4).transpose(1, 0, 2).reshape(P, GROUPS * 4)
        dl_mat = dlv.reshape(GROUPS, P).astype(np.int32)
        oht = (dl_mat[:, :, None] == np.arange(P, dtype=np.int32)[None, None, :])
        oht = np.ascontiguousarray(
            oht.transpose(1, 0, 2).reshape(P, GROUPS * 128)
        ).astype(mybir.dt.np(mybir.dt.float8e4))

        in_maps.append(
            dict(
                tbl=tbl, idx_g=np.ascontiguousarray(idx_g),
                es4=np.ascontiguousarray(es4).astype(BF16),
                sh_t=np.ascontiguousarray(sh_t).astype(BF16),
                oht=oht,
                w1t=w1t.astype(BF16), w2t=w2t.astype(BF16),
            )
        )
    return in_maps, orders


def _compute_schedule(edge_dst):
    dst_all = np.asarray(edge_dst).astype(np.int64)
    cnt = np.zeros((NCORES, NB), np.int64)
    for c in range(NCORES):
        dl = dst_all[(dst_all // NODES_PER_CORE) == c] - c * NODES_PER_CORE
        cnt[c] = np.bincount(dl >> 7, minlength=NB)
    scnt = -np.sort(-cnt, axis=1)                  # each core's blocks desc
    ng_k = np.maximum(np.ceil(scnt.max(axis=0) / 128.0).astype(np.int64), 1)
    while int(ng_k.sum()) % 4:
        ng_k[-1] += 1
    return tuple(int(x) for x in ng_k)


def kernel(node_features, edge_src, edge_dst, edge_sh, edge_scalars, fc_w1, fc_w2):
    node_features = np.asarray(node_features, dtype=np.float32)
    edge_sh = np.asarray(edge_sh, dtype=np.float32)
    edge_scalars = np.asarray(edge_scalars, dtype=np.float32)
    fc_w1 = np.asarray(fc_w1, dtype=np.float32)
    fc_w2 = np.asarray(fc_w2, dtype=np.float32)

    schedule = _compute_schedule(edge_dst)
    if schedule not in _PROG_CACHE:
        _PROG_CACHE[schedule] = _build_program(schedule)
    nc = _PROG_CACHE[schedule]

    in_maps, orders = _prep(
        node_features, edge_src, edge_dst, edge_sh, edge_scalars, fc_w1, fc_w2,
        schedule,
    )
    res = run_bass_kernel_spmd(nc, in_maps, core_ids=list(range(NCORES)))
    outs = []
    for c in range(NCORES):
        no = res.results[c]["nodeout"].reshape(NB, 128, 32)
        oc = np.empty_like(no)
        oc[orders[c]] = no
        outs.append(oc.reshape(NODES_PER_CORE, 32))
    out = np.concatenate(outs, axis=0)
    return out[:N_NODES].astype(np.float32)


# revision 18
# speedup vs baseline: 1.0668x; 1.0629x over previous
"""Trainium2 Bass kernel for nn_Convolution (e3nn-style GNN message passing).

Strategy (8 NeuronCores, SPMD, no collectives):
- Sort edges by destination; core c owns destination nodes [6400c, 6400(c+1)).
- Per core: edges are binned into 50 node-blocks (128 nodes each) and padded to
  NG groups of 128 edges per block. Dummy edges gather a zero table row, so
  every tensor-product output term (all bilinear in source features) is 0.
- Gather source features with dma_gather from a 256B-padded bf16 table, one
  512-idx call per window.
- Radial MLP layer 1 on PE with tile_position row-packed K=8 matmuls,
  layer 2 per-group with h as the stationary operand (w lands [edge, 256]).
- All matmuls and DVE product ops in bf16 (PSUM accumulates fp32); fp32 only
  for PSUM accumulators and the final staged output.
- TP products on DVE via broadcast APs; the per-edge contraction over u is
  DEFERRED into the scatter matmul: one-hot(dst) x [512-wide product tile]
  accumulates in PSUM over each block, reduced over u once per block.
"""

import math
import os
import numpy as np
from ml_dtypes import bfloat16 as BF16

_TRACE_SIM = bool(int(os.environ.get('K_TRACE_SIM', '0')))
_NO_GATHER = bool(int(os.environ.get('K_NO_GATHER', '0')))
_NO_TP = bool(int(os.environ.get('K_NO_TP', '0')))
_NO_MM = bool(int(os.environ.get('K_NO_MM', '0')))


import concourse.bass as bass
import concourse.bacc as bacc
import concourse.mybir as mybir
from concourse.tile import TileContext
from concourse.bass_utils import run_bass_kernel_spmd

# ---------------- problem constants (hardcoded per spec) ----------------
N_NODES, N_EDGES, NUM_BASIS, HIDDEN = 50000, 800000, 8, 256
MUL = 8
INV_SQRT3 = float(1.0 / np.sqrt(3.0))
A_SCALAR = float(np.sqrt(1.0 / 128.0))
A_VECTOR = float(np.sqrt(3.0 / 128.0))
SQRT2 = float(np.sqrt(2.0))
DEG_SCALE = float(1.0 / np.sqrt(N_EDGES / N_NODES))

NCORES = 8
P = 128
NODES_PER_CORE = 6400          # 50 blocks of 128; 8*6400 = 51200 >= 50000
NB = 50                        # node blocks per core
# table: rows 1..50000 = nodes 0..49999; row 50001 = zeros (dummy target).
# gather base = row 32768, int16 idx = node - 32767 in [-32767, 17232];
# dummy idx = +17233 (always non-negative so it never hits the trailing-
# negative trim). Each gather's last (trim-order) index is forced >= 0 by an
# in-block edge swap on the host.
TBL_ROWS = 50004
TBL_COLS = 128                 # bf16 -> 256B rows (dma_gather elem granule)
GBASE = 32768
DUMMY_IDX = 50001 - GBASE

_PROG_CACHE = {}


def _dma_gather_thin(g, out_ap, in_ap, idxs_ap, num_idxs, elem_size, elem_step,
                     queue_num):
    """dma_gather variant with elem_size < 256B (table stride stays 256B)."""
    from concourse.bass import exact_div, round_up_to_multiple
    assert idxs_ap.dtype == mybir.dt.int16
    assert in_ap.dtype == out_ap.dtype
    assert in_ap.ap[-1][1] == out_ap.ap[-1][1] == elem_size
    assert out_ap.ap[0][1] * out_ap.ap[1][1] == round_up_to_multiple(num_idxs, 128)
    assert in_ap.ap[0][0] == elem_step
    stride_bytes = elem_step * mybir.dt.size(in_ap.dtype)
    return g.add_instruction(
        mybir.InstDMAGatherAnt(
            name=g.bass.get_next_instruction_name(),
            ins=[*g.lower_ap_dma(in_ap, for_custom_bir_dma=True),
                 g.lower_ap(idxs_ap),
                 g.lower_val_access(g.to_reg(num_idxs))],
            outs=[g.lower_ap(out_ap)],
            transpose=False,
            num_idxs=num_idxs,
            elem_size=elem_size,
            stride_bytes_256=exact_div(stride_bytes, 256),
            gen_mode=0,
            single_packet=True,
            queue_num=queue_num,
            sbuf_tokens_per_rank=0,
            sbuf_free_dim_per_rank=0,
            sbuf_free_dim_pad_per_rank=0,
            sbuf_byte_offset=0,
        )
    )


# ---------------- device program ----------------
def _build_program(schedule):
    GROUPS = sum(schedule)
    WINDOWS = GROUPS // 4            # 4 groups (512 edges) per window
    NSUPER = (WINDOWS + 1) // 2      # one 1024-idx gather per 2 windows
    NJ = (WINDOWS + 3) // 4          # es_w4 column blocks
    ES_CHUNK_J = 3                   # es col-blocks per streamed chunk
    NCHUNK = (NJ + ES_CHUNK_J - 1) // ES_CHUNK_J

    nc = bacc.Bacc(num_devices=NCORES, num_swdge_queues=4)
    f32, i16 = mybir.dt.float32, mybir.dt.int16
    bf16 = mybir.dt.bfloat16

    tbl = nc.dram_tensor("tbl", [TBL_ROWS, TBL_COLS], bf16, kind="ExternalInput")
    idx_g = nc.dram_tensor("idx_g", [P, GROUPS * 8], i16, kind="ExternalInput")
    es4 = nc.dram_tensor("es4", [32, NJ * 512], bf16, kind="ExternalInput")
    sh_t = nc.dram_tensor("sh_t", [P, GROUPS * 4], bf16, kind="ExternalInput")
    oht = nc.dram_tensor("oht", [P, GROUPS * 128], mybir.dt.float8e4,
                         kind="ExternalInput")
    w1t = nc.dram_tensor("w1t", [P, 256], bf16, kind="ExternalInput")
    w2t = nc.dram_tensor("w2t", [P, 512], bf16, kind="ExternalInput")
    nodeout = nc.dram_tensor("nodeout", [NODES_PER_CORE, 32], f32, kind="ExternalOutput")

    gmeta = []                       # group -> (block slot k, gib, ngk)
    for k, ngk in enumerate(schedule):
        for gib in range(ngk):
            gmeta.append((k, gib, ngk))
    assert len(gmeta) == GROUPS

    AX = mybir.AxisListType.X
    ADD = mybir.AluOpType.add
    MUL_ = mybir.AluOpType.mult
    EQ = mybir.AluOpType.is_equal
    RELU = mybir.ActivationFunctionType.Relu

    with TileContext(nc, trace_sim=_TRACE_SIM) as tc:
        with tc.tile_pool(name="const", bufs=1) as cpool, \
             tc.tile_pool(name="stream", bufs=2) as spool, \
             tc.tile_pool(name="gat", bufs=8) as gpool, \
             tc.tile_pool(name="ohp", bufs=4) as ohpool, \
             tc.tile_pool(name="work", bufs=3) as wpool, \
             tc.tile_pool(name="psum", bufs=2, space="PSUM") as pp, \
             tc.tile_pool(name="psum1", bufs=1, space="PSUM") as pp1:

            # constants resident in SBUF
            ig_sb = cpool.tile([P, GROUPS * 8], i16, name="ig")
            nc.sync.dma_start(ig_sb[:], idx_g[:])
            sh_sb = cpool.tile([P, GROUPS, 4], bf16, name="sh")
            nc.sync.dma_start(sh_sb[:], sh_t[:].rearrange("p (g k) -> p g k", k=4))
            w1_sb = cpool.tile([P, 256], bf16, name="w1")
            nc.sync.dma_start(w1_sb[:], w1t[:])
            w2_sb = cpool.tile([P, 2, 256], bf16, name="w2")
            nc.sync.dma_start(w2_sb[:], w2t[:].rearrange("p (h n) -> p h n", h=2))

            state = {"acc": None}

            def emit_scatter(g0, oh4, scat):
                for gg in range(4):
                    g = g0 + gg
                    b, gib, ngk = gmeta[g]
                    if gib == 0:
                        state["acc"] = pp.tile(
                            [P, 512], f32, space="PSUM", tag="acc", name="acc"
                        )
                    acc_ps = state["acc"]
                    nc.tensor.matmul(
                        acc_ps[:],
                        lhsT=oh4[:, gg, :], rhs=scat[:, gg, :],
                        start=(gib == 0), stop=(gib == ngk - 1),
                    )
                    if gib == ngk - 1:
                        stage = wpool.tile([P, 32], f32, tag="stage",
                                           name="stage")
                        nc.vector.tensor_reduce(
                            out=stage[:, 0:8],
                            in_=acc_ps[:, 0:128].rearrange(
                                "p (u w) -> p w u", u=16
                            ),
                            axis=AX, op=ADD,
                        )
                        nc.vector.tensor_reduce(
                            out=stage[:, 8:32],
                            in_=acc_ps[:, 128:512].rearrange(
                                "p (u wi) -> p wi u", u=16
                            ),
                            axis=AX, op=ADD,
                        )
                        nc.sync.dma_start(
                            nodeout[128 * b : 128 * b + 128, :], stage[:]
                        )

            pending = None
            for w in range(WINDOWS):
                c = w % 4
                j = w // 4
                g0 = 4 * w

                # --- stream es chunk (every ES_CHUNK_J col-blocks)
                if j % ES_CHUNK_J == 0:
                    jw = min(ES_CHUNK_J, NJ - j)
                    es_sb = spool.tile([P, ES_CHUNK_J * 512], bf16, tag="es")
                    for cc in range(4):
                        nc.sync.dma_start(
                            es_sb[32 * cc : 32 * cc + 8, : jw * 512],
                            es4[8 * cc : 8 * cc + 8, j * 512 : (j + jw) * 512],
                        )
                jj = j % ES_CHUNK_J

                # --- stream this window's 4 one-hot scatter matrices (fp8),
                # upcast to bf16 on the Act engine (0/1 are exact in fp8)
                oh4f = ohpool.tile([P, 4, P], mybir.dt.float8e4, tag="oh4f")
                nc.sync.dma_start(
                    oh4f[:], oht[:, g0 * 128 : (g0 + 4) * 128].rearrange(
                        "p (g n) -> p g n", g=4
                    )
                )
                oh4 = wpool.tile([P, 4, P], bf16, tag="oh4b")
                nc.scalar.copy(out=oh4[:], in_=oh4f[:])

                # --- gather: one 1024-idx call per 2 windows, cycling queues
                if w % 2 == 0:
                    sgi = w // 2
                    nidx = 1024 if w + 1 < WINDOWS else 512
                    x_sup = gpool.tile([P, 8, 32], bf16, tag="xc")
                    _dma_gather_thin(
                        nc.gpsimd,
                        out_ap=x_sup[:, : nidx // 128, :], in_ap=tbl[GBASE:, :32],
                        idxs_ap=ig_sb[:, sgi * 64 : sgi * 64 + nidx // 16],
                        num_idxs=nidx, elem_size=32, elem_step=TBL_COLS,
                        queue_num=sgi % 4,
                    )
                x_c = x_sup[:, 4 * (w % 2) : 4 * (w % 2) + 4, :]
                # --- MLP1: h[comp, edge] for 512 edges, two 128-comp halves
                h_ps = pp1.tile([P, 2, 512], f32, space="PSUM", tag="hps")
                for half in range(2):
                    nc.tensor.matmul(
                        h_ps[:, half, :],
                        lhsT=w1_sb[32 * c : 32 * c + 8, half * 128 : half * 128 + 128],
                        rhs=es_sb[32 * c : 32 * c + 8, jj * 512 : jj * 512 + 512],
                        start=True, stop=True,
                        tile_position=(32 * c, 0),
                    )
                h_sb = wpool.tile([P, 2, 512], bf16, tag="hsb")
                for half in range(2):
                    nc.scalar.activation(
                        out=h_sb[:, half, :], in_=h_ps[:, half, :], func=RELU
                    )

                # --- MLP2 per group: w[edge, 256] in PSUM
                w_ps = pp.tile([P, 4, 256], f32, space="PSUM", tag="wps")
                for gg in range(4):
                    for half in range(2):
                        nc.tensor.matmul(
                            w_ps[:, gg, :],
                            lhsT=h_sb[:, half, gg * 128 : gg * 128 + 128],
                            rhs=w2_sb[:, half, :],
                            start=(half == 0), stop=(half == 1),
                        )

                # --- TP products (batched over the 4 groups)
                xs = x_c                              # [P, 4, 128] (use cols 0:32)
                shw = sh_sb[:, g0 : g0 + 4, :]        # [P, 4, 4]
                scat = wpool.tile([P, 4, 512], bf16, tag="scat")
                ab16 = wpool.tile([P, 4, 16], bf16, tag="ab16")
                # a[u] = s1[u] * s2
                nc.vector.tensor_tensor(
                    out=ab16[:, :, 0:8], in0=xs[:, :, 0:8],
                    in1=shw[:, :, 0:1].to_broadcast([P, 4, 8]), op=MUL_,
                )
                # b[u] = sum_i v1[u,i] * v2[i]
                pb = wpool.tile([P, 4, 8, 3], bf16, tag="pb")
                nc.vector.tensor_tensor(
                    out=pb[:],
                    in0=xs[:, :, 8:32].rearrange("p g (u i) -> p g u i", u=8),
                    in1=shw[:, :, 1:4].unsqueeze(2).to_broadcast([P, 4, 8, 3]),
                    op=MUL_,
                )
                with nc.allow_low_precision(reason="3-elem dot, tol 2e-2"):
                    nc.vector.tensor_reduce(
                        out=ab16[:, :, 8:16], in_=pb[:], axis=AX, op=ADD
                    )
                # ps = ab16[u'] * w01[u', w_]  -> scat[:, :, 0:128]
                nc.vector.tensor_tensor(
                    out=scat[:, :, 0:128].rearrange("p g (u w) -> p g u w", u=16),
                    in0=ab16[:].unsqueeze(3).to_broadcast([P, 4, 16, 8]),
                    in1=w_ps[:, :, 0:128].rearrange("p g (u w) -> p g u w", u=16),
                    op=MUL_,
                )
                # z23[t,u,i]: t=0 -> s1[u]*v2[i], t=1 -> v1[u,i]*s2
                z23 = wpool.tile([P, 4, 2, 24], bf16, tag="z23")
                nc.vector.tensor_tensor(
                    out=z23[:, :, 0, :].rearrange("p g (u i) -> p g u i", u=8),
                    in0=xs[:, :, 0:8].unsqueeze(3).to_broadcast([P, 4, 8, 3]),
                    in1=shw[:, :, 1:4].unsqueeze(2).to_broadcast([P, 4, 8, 3]),
                    op=MUL_,
                )
                nc.vector.tensor_tensor(
                    out=z23[:, :, 1, :], in0=xs[:, :, 8:32],
                    in1=shw[:, :, 0:1].to_broadcast([P, 4, 24]), op=MUL_,
                )
                # scat cols 128:512 ((t,u,w,i)) = z23[t,u,i] * w23[t,u,w]
                nc.vector.tensor_tensor(
                    out=scat[:, :, 128:512].rearrange(
                        "p g (t u w i) -> p g t u w i", t=2, u=8, w=8
                    ),
                    in0=z23[:].rearrange("p g t (u i) -> p g t u i", u=8)
                        .unsqueeze(4).to_broadcast([P, 4, 2, 8, 8, 3]),
                    in1=w_ps[:, :, 128:256]
                        .rearrange("p g (t u w) -> p g t u w", t=2, u=8, w=8)
                        .unsqueeze(5).to_broadcast([P, 4, 2, 8, 8, 3]),
                    op=MUL_,
                )

                emit_scatter(g0, oh4, scat)
    nc.compile()
    return nc


# ---------------- host-side prep ----------------
def _prep(node_features, edge_src, edge_dst, edge_sh, edge_scalars, fc_w1, fc_w2,
          schedule):
    GROUPS = sum(schedule)
    EPAD = GROUPS * 128
    Gq = np.zeros(NB + 1, np.int64)
    Gq[1:] = np.cumsum(schedule)
    Gedge = Gq * 128                  # edge-slot start of block slot k
    WINDOWS = GROUPS // 4
    NJ = (WINDOWS + 3) // 4
    # gather call boundaries: 1024-idx supers, possibly a 512 tail
    call_ends = []
    wpos = 0
    while wpos < WINDOWS:
        nidx = 1024 if wpos + 1 < WINDOWS else 512
        call_ends.append(wpos * 512 + nidx)
        wpos += 2

    # fold all scalar coefficients into the weights
    w1s = (fc_w1 * (1.0 / math.sqrt(NUM_BASIS))).astype(np.float32)     # [8, 256]
    w2 = (fc_w2 * (SQRT2 / math.sqrt(HIDDEN))).astype(np.float64)       # [256, 256]
    w2 = w2.reshape(HIDDEN, 4, MUL, MUL)
    coef = np.array(
        [A_SCALAR, A_SCALAR * INV_SQRT3, A_VECTOR * INV_SQRT3, A_VECTOR * INV_SQRT3]
    ) * DEG_SCALE
    w2 = w2 * coef[None, :, None, None]
    # device col order: [w01 (u'16, w8) | w2 (u8, w8) | w3 (u8, w8)]
    w2dev = np.concatenate(
        [
            w2[:, 0].reshape(HIDDEN, 64),
            w2[:, 1].reshape(HIDDEN, 64),
            w2[:, 2].reshape(HIDDEN, 64),
            w2[:, 3].reshape(HIDDEN, 64),
        ],
        axis=1,
    ).astype(np.float32)                                                # [256, 256]

    w1t = np.zeros((P, 256), np.float32)
    for c in range(4):
        w1t[32 * c : 32 * c + 8] = w1s
    w2t = np.zeros((P, 512), np.float32)
    w2t[:, 0:256] = w2dev[0:128]
    w2t[:, 256:512] = w2dev[128:256]
    tbl = np.zeros((TBL_ROWS, TBL_COLS), BF16)
    tbl[1 : N_NODES + 1, 0:32] = node_features

    src_all = np.asarray(edge_src).astype(np.int64)
    dst_all = np.asarray(edge_dst).astype(np.int64)
    es_all = np.asarray(edge_scalars).astype(np.float32)
    sh_all = np.asarray(edge_sh).astype(np.float32)
    core_of = dst_all // NODES_PER_CORE

    in_maps = []
    orders = []
    for cid in range(NCORES):
        sel = np.nonzero(core_of == cid)[0]
        d = dst_all[sel]
        order = np.argsort(d, kind="stable")
        sel = sel[order]
        d = d[order]
        blk = (d - NODES_PER_CORE * cid) >> 7
        cnt = np.bincount(blk, minlength=NB)
        border = np.argsort(-cnt, kind="stable")   # block id for slot k
        orders.append(border)
        kpos = np.empty(NB, np.int64)
        kpos[border] = np.arange(NB)
        assert (cnt[border] <= np.asarray(schedule) * 128).all(), cid
        start = np.zeros(NB, np.int64)
        start[1:] = np.cumsum(cnt)[:-1]
        rank = np.arange(len(sel)) - start[blk]
        slot = Gedge[kpos[blk]] + rank

        srcv = np.full(EPAD, -1, np.int64)
        srcv[slot] = src_all[sel]
        shv = np.zeros((EPAD, 4), np.float32)
        shv[slot] = sh_all[sel]
        esv = np.zeros((EPAD, 8), np.float32)
        esv[slot] = es_all[sel]
        dlv = np.zeros(EPAD, np.float32)
        dlv[slot] = (d - NODES_PER_CORE * cid - 128 * blk).astype(np.float32)

        # --- gather indices: idx = node - (GBASE - 1 - 1)... row = node+1,
        # idx = row - GBASE = node + 1 - GBASE; dummy -> DUMMY_IDX (>= 0)
        idxv = np.where(srcv >= 0, srcv + 1 - GBASE, DUMMY_IDX).astype(np.int64)
        # force the trim-order-last index of each 512-idx gather call to be
        # >= 0 by swapping that edge with a non-negative-idx edge of the SAME
        # node block (any within-block permutation is valid).
        ends = set(call_ends)
        for jl in call_ends:
            jl = jl - 1
            if idxv[jl] >= 0:
                continue
            k0 = int(np.searchsorted(Gedge, jl, side="right")) - 1
            lo, hi = int(Gedge[k0]), int(Gedge[k0 + 1])
            cand = np.nonzero(idxv[lo:hi] >= 0)[0]
            # exclude other calls' final slots
            cand = [lo + q for q in cand if (lo + q + 1) not in ends]
            assert cand, "no swap candidate in block"
            q = cand[0]
            for arr in (idxv, srcv, dlv):
                arr[jl], arr[q] = arr[q], arr[jl]
            for arr in (shv, esv):
                tmpq = arr[q].copy()
                arr[q] = arr[jl]
                arr[jl] = tmpq
        idx_g = np.tile(
            idxv.reshape(-1, 16).T.astype(np.int16), (8, 1)
        )  # wrap is uniform: IDXW*NSUPER cols total

        # es4: window w at rows 32*(w%4)+b, cols [ (w//4)*512, +512 )
        es4 = np.zeros((32, NJ * 512), np.float32)
        esw = esv.reshape(WINDOWS, 512, 8)
        for c in range(4):
            wsel = np.arange(c, WINDOWS, 4)       # these windows use strip c
            nw = len(wsel)                        # w//4 == index within wsel
            es4[8 * c : 8 * c + 8, : nw * 512] = (
                esw[wsel].transpose(2, 0, 1).reshape(8, nw * 512)
            )

        sh_t = shv.reshape(GROUPS, P, 4).transpose(1, 0, 2).reshape(P, GROUPS * 4)
        dl_mat = dlv.reshape(GROUPS, P).astype(np.int32)
        oht = (dl_mat[:, :, None] == np.arange(P, dtype=np.int32)[None, None, :])
        oht = np.ascontiguousarray(
            oht.transpose(1, 0, 2).reshape(P, GROUPS * 128)
        ).astype(mybir.dt.np(mybir.dt.float8e4))

        in_maps.append(
            dict(
                tbl=tbl, idx_g=np.ascontiguousarray(idx_g),
                es4=np.ascontiguousarray(es4).astype(BF16),
                sh_t=np.ascontiguousarray(sh_t).astype(BF16),
                oht=oht,
                w1t=w1t.astype(BF16), w2t=w2t.astype(BF16),
            )
        )
    return in_maps, orders


def _compute_schedule(edge_dst):
    dst_all = np.asarray(edge_dst).astype(np.int64)
    cnt = np.zeros((NCORES, NB), np.int64)
    for c in range(NCORES):
        dl = dst_all[(dst_all // NODES_PER_CORE) == c] - c * NODES_PER_CORE
        cnt[c] = np.bincount(dl >> 7, minlength=NB)
    scnt = -np.sort(-cnt, axis=1)                  # each core's blocks desc
    ng_k = np.maximum(np.ceil(scnt.max(axis=0) / 128.0).astype(np.int64), 1)
    while int(ng_k.sum()) % 4:
        ng_k[-1] += 1
    return tuple(int(x) for x in ng_k)


def kernel(node_features, edge_src, edge_dst, edge_sh, edge_scalars, fc_w1, fc_w2):
    node_features = np.asarray(node_features, dtype=np.float32)
    edge_sh = np.asarray(edge_sh, dtype=np.float32)
    edge_scalars = np.asarray(edge_scalars, dtype=np.float32)
    fc_w1 = np.asarray(fc_w1, dtype=np.float32)
    fc_w2 = np.asarray(fc_w2, dtype=np.float32)

    schedule = _compute_schedule(edge_dst)
    if schedule not in _PROG_CACHE:
        _PROG_CACHE[schedule] = _build_program(schedule)
    nc = _PROG_CACHE[schedule]

    in_maps, orders = _prep(
        node_features, edge_src, edge_dst, edge_sh, edge_scalars, fc_w1, fc_w2,
        schedule,
    )
    res = run_bass_kernel_spmd(nc, in_maps, core_ids=list(range(NCORES)))
    outs = []
    for c in range(NCORES):
        no = res.results[c]["nodeout"].reshape(NB, 128, 32)
        oc = np.empty_like(no)
        oc[orders[c]] = no
        outs.append(oc.reshape(NODES_PER_CORE, 32))
    out = np.concatenate(outs, axis=0)
    return out[:N_NODES].astype(np.float32)


# revision 19
# speedup vs baseline: 1.1990x; 1.1239x over previous
"""Trainium2 Bass kernel for nn_Convolution (e3nn-style GNN message passing).

Strategy (8 NeuronCores, SPMD, no collectives):
- Sort edges by destination; core c owns destination nodes [6400c, 6400(c+1)).
- Per core: edges are binned into 50 node-blocks (128 nodes each) and padded to
  NG groups of 128 edges per block. Dummy edges gather a zero table row, so
  every tensor-product output term (all bilinear in source features) is 0.
- Gather source features with dma_gather from a 256B-padded bf16 table, one
  512-idx call per window.
- Radial MLP layer 1 on PE with tile_position row-packed K=8 matmuls,
  layer 2 per-group with h as the stationary operand (w lands [edge, 256]).
- All matmuls and DVE product ops in bf16 (PSUM accumulates fp32); fp32 only
  for PSUM accumulators and the final staged output.
- TP products on DVE via broadcast APs; the per-edge contraction over u is
  DEFERRED into the scatter matmul: one-hot(dst) x [512-wide product tile]
  accumulates in PSUM over each block, reduced over u once per block.
"""

import math
import os
import numpy as np
from ml_dtypes import bfloat16 as BF16

_TRACE_SIM = bool(int(os.environ.get('K_TRACE_SIM', '0')))
_NO_GATHER = bool(int(os.environ.get('K_NO_GATHER', '0')))
_NO_TP = bool(int(os.environ.get('K_NO_TP', '0')))
_NO_MM = bool(int(os.environ.get('K_NO_MM', '0')))


import concourse.bass as bass
import concourse.bacc as bacc
import concourse.mybir as mybir
from concourse.tile import TileContext
from concourse.bass_utils import run_bass_kernel_spmd

# ---------------- problem constants (hardcoded per spec) ----------------
N_NODES, N_EDGES, NUM_BASIS, HIDDEN = 50000, 800000, 8, 256
MUL = 8
INV_SQRT3 = float(1.0 / np.sqrt(3.0))
A_SCALAR = float(np.sqrt(1.0 / 128.0))
A_VECTOR = float(np.sqrt(3.0 / 128.0))
SQRT2 = float(np.sqrt(2.0))
DEG_SCALE = float(1.0 / np.sqrt(N_EDGES / N_NODES))

NCORES = 8
P = 128
NODES_PER_CORE = 6400          # 50 blocks of 128; 8*6400 = 51200 >= 50000
NB = 50                        # node blocks per core
# table: rows 1..50000 = nodes 0..49999; row 50001 = zeros (dummy target).
# gather base = row 32768, int16 idx = node - 32767 in [-32767, 17232];
# dummy idx = +17233 (always non-negative so it never hits the trailing-
# negative trim). Each gather's last (trim-order) index is forced >= 0 by an
# in-block edge swap on the host.
TBL_ROWS = 50004
TBL_COLS = 128                 # bf16 -> 256B rows (dma_gather elem granule)
GBASE = 32768
DUMMY_IDX = 50001 - GBASE

_PROG_CACHE = {}


def _dma_gather_thin(g, out_ap, in_ap, idxs_ap, num_idxs, elem_size, elem_step,
                     queue_num):
    """dma_gather variant with elem_size < 256B (table stride stays 256B)."""
    from concourse.bass import exact_div, round_up_to_multiple
    assert idxs_ap.dtype == mybir.dt.int16
    assert in_ap.dtype == out_ap.dtype
    assert in_ap.ap[-1][1] == out_ap.ap[-1][1] == elem_size
    assert out_ap.ap[0][1] * out_ap.ap[1][1] == round_up_to_multiple(num_idxs, 128)
    assert in_ap.ap[0][0] == elem_step
    stride_bytes = elem_step * mybir.dt.size(in_ap.dtype)
    return g.add_instruction(
        mybir.InstDMAGatherAnt(
            name=g.bass.get_next_instruction_name(),
            ins=[*g.lower_ap_dma(in_ap, for_custom_bir_dma=True),
                 g.lower_ap(idxs_ap),
                 g.lower_val_access(g.to_reg(num_idxs))],
            outs=[g.lower_ap(out_ap)],
            transpose=False,
            num_idxs=num_idxs,
            elem_size=elem_size,
            stride_bytes_256=exact_div(stride_bytes, 256),
            gen_mode=0,
            single_packet=True,
            queue_num=queue_num,
            sbuf_tokens_per_rank=0,
            sbuf_free_dim_per_rank=0,
            sbuf_free_dim_pad_per_rank=0,
            sbuf_byte_offset=0,
        )
    )


# ---------------- device program ----------------
def _build_program(schedule):
    GROUPS = sum(schedule)
    WINDOWS = GROUPS // 4            # 4 groups (512 edges) per window
    NSUPER = (WINDOWS + 1) // 2      # one 1024-idx gather per 2 windows
    NJ = (WINDOWS + 3) // 4          # es_w4 column blocks
    ES_CHUNK_J = 3                   # es col-blocks per streamed chunk
    NCHUNK = (NJ + ES_CHUNK_J - 1) // ES_CHUNK_J

    nc = bacc.Bacc(num_devices=NCORES, num_swdge_queues=4)
    f32, i16 = mybir.dt.float32, mybir.dt.int16
    bf16 = mybir.dt.bfloat16

    tbl = nc.dram_tensor("tbl", [TBL_ROWS, TBL_COLS], bf16, kind="ExternalInput")
    idx_g = nc.dram_tensor("idx_g", [P, GROUPS * 8], i16, kind="ExternalInput")
    es4 = nc.dram_tensor("es4", [32, NJ * 512], bf16, kind="ExternalInput")
    sh_t = nc.dram_tensor("sh_t", [P, GROUPS * 4], bf16, kind="ExternalInput")
    oht = nc.dram_tensor("oht", [P, GROUPS * 128], mybir.dt.float8e4,
                         kind="ExternalInput")
    w1t = nc.dram_tensor("w1t", [P, 256], bf16, kind="ExternalInput")
    w2t = nc.dram_tensor("w2t", [P, 512], bf16, kind="ExternalInput")
    nodeout = nc.dram_tensor("nodeout", [NODES_PER_CORE, 32], f32, kind="ExternalOutput")

    gmeta = []                       # group -> (block slot k, gib, ngk)
    for k, ngk in enumerate(schedule):
        for gib in range(ngk):
            gmeta.append((k, gib, ngk))
    assert len(gmeta) == GROUPS

    AX = mybir.AxisListType.X
    ADD = mybir.AluOpType.add
    MUL_ = mybir.AluOpType.mult
    EQ = mybir.AluOpType.is_equal
    RELU = mybir.ActivationFunctionType.Relu

    with TileContext(nc, trace_sim=_TRACE_SIM) as tc:
        with tc.tile_pool(name="const", bufs=1) as cpool, \
             tc.tile_pool(name="stream", bufs=3) as spool, \
             tc.tile_pool(name="gat", bufs=8) as gpool, \
             tc.tile_pool(name="ohp", bufs=4) as ohpool, \
             tc.tile_pool(name="work", bufs=3) as wpool, \
             tc.tile_pool(name="psum", bufs=2, space="PSUM") as pp, \
             tc.tile_pool(name="psum1", bufs=1, space="PSUM") as pp1:

            # constants resident in SBUF
            ig_sb = cpool.tile([P, GROUPS * 8], i16, name="ig")
            nc.sync.dma_start(ig_sb[:], idx_g[:])
            sh_sb = cpool.tile([P, GROUPS, 4], bf16, name="sh")
            nc.sync.dma_start(sh_sb[:], sh_t[:].rearrange("p (g k) -> p g k", k=4))
            w1_sb = cpool.tile([P, 256], bf16, name="w1")
            nc.sync.dma_start(w1_sb[:], w1t[:])
            w2_sb = cpool.tile([P, 2, 256], bf16, name="w2")
            nc.sync.dma_start(w2_sb[:], w2t[:].rearrange("p (h n) -> p h n", h=2))

            state = {"acc": None}

            def emit_scatter(g0, oh4, scat):
                for gg in range(4):
                    g = g0 + gg
                    b, gib, ngk = gmeta[g]
                    if gib == 0:
                        state["acc"] = pp.tile(
                            [P, 512], f32, space="PSUM", tag="acc", name="acc"
                        )
                    acc_ps = state["acc"]
                    nc.tensor.matmul(
                        acc_ps[:],
                        lhsT=oh4[:, gg, :], rhs=scat[:, gg, :],
                        start=(gib == 0), stop=(gib == ngk - 1),
                    )
                    if gib == ngk - 1:
                        stage = wpool.tile([P, 32], f32, tag="stage",
                                           name="stage")
                        nc.vector.tensor_reduce(
                            out=stage[:, 0:8],
                            in_=acc_ps[:, 0:128].rearrange(
                                "p (u w) -> p w u", u=16
                            ),
                            axis=AX, op=ADD,
                        )
                        nc.vector.tensor_reduce(
                            out=stage[:, 8:32],
                            in_=acc_ps[:, 128:512].rearrange(
                                "p (u wi) -> p wi u", u=16
                            ),
                            axis=AX, op=ADD,
                        )
                        nc.sync.dma_start(
                            nodeout[128 * b : 128 * b + 128, :], stage[:]
                        )

            pending = None
            for w in range(WINDOWS):
                c = w % 4
                j = w // 4
                g0 = 4 * w

                # --- stream es chunk (every ES_CHUNK_J col-blocks)
                if j % ES_CHUNK_J == 0:
                    jw = min(ES_CHUNK_J, NJ - j)
                    es_sb = spool.tile([P, ES_CHUNK_J * 512], bf16, tag="es")
                    for cc in range(4):
                        nc.sync.dma_start(
                            es_sb[32 * cc : 32 * cc + 8, : jw * 512],
                            es4[8 * cc : 8 * cc + 8, j * 512 : (j + jw) * 512],
                        )
                jj = j % ES_CHUNK_J

                # --- stream this window's 4 one-hot scatter matrices (fp8),
                # upcast to bf16 on the Act engine (0/1 are exact in fp8)
                oh4f = ohpool.tile([P, 4, P], mybir.dt.float8e4, tag="oh4f")
                nc.sync.dma_start(
                    oh4f[:], oht[:, g0 * 128 : (g0 + 4) * 128].rearrange(
                        "p (g n) -> p g n", g=4
                    )
                )
                oh4 = wpool.tile([P, 4, P], bf16, tag="oh4b")
                nc.scalar.copy(out=oh4[:], in_=oh4f[:])

                # --- gather: one 1024-idx call per 2 windows, cycling queues
                if w % 2 == 0:
                    sgi = w // 2
                    nidx = 1024 if w + 1 < WINDOWS else 512
                    x_sup = gpool.tile([P, 8, 32], bf16, tag="xc")
                    _dma_gather_thin(
                        nc.gpsimd,
                        out_ap=x_sup[:, : nidx // 128, :], in_ap=tbl[GBASE:, :32],
                        idxs_ap=ig_sb[:, sgi * 64 : sgi * 64 + nidx // 16],
                        num_idxs=nidx, elem_size=32, elem_step=TBL_COLS,
                        queue_num=sgi % 4,
                    )
                x_c = x_sup[:, 4 * (w % 2) : 4 * (w % 2) + 4, :]
                # --- MLP1: h[comp, edge] for 512 edges, two 128-comp halves
                h_ps = pp1.tile([P, 2, 512], f32, space="PSUM", tag="hps")
                for half in range(2):
                    nc.tensor.matmul(
                        h_ps[:, half, :],
                        lhsT=w1_sb[32 * c : 32 * c + 8, half * 128 : half * 128 + 128],
                        rhs=es_sb[32 * c : 32 * c + 8, jj * 512 : jj * 512 + 512],
                        start=True, stop=True,
                        tile_position=(32 * c, 0),
                    )
                h_sb = wpool.tile([P, 2, 512], bf16, tag="hsb")
                for half in range(2):
                    nc.scalar.activation(
                        out=h_sb[:, half, :], in_=h_ps[:, half, :], func=RELU
                    )

                # --- MLP2 per group: w[edge, 256] in PSUM
                w_ps = pp.tile([P, 4, 256], f32, space="PSUM", tag="wps")
                for gg in range(4):
                    for half in range(2):
                        nc.tensor.matmul(
                            w_ps[:, gg, :],
                            lhsT=h_sb[:, half, gg * 128 : gg * 128 + 128],
                            rhs=w2_sb[:, half, :],
                            start=(half == 0), stop=(half == 1),
                        )

                # --- TP products (batched over the 4 groups)
                xs = x_c                              # [P, 4, 128] (use cols 0:32)
                shw = sh_sb[:, g0 : g0 + 4, :]        # [P, 4, 4]
                scat = wpool.tile([P, 4, 512], bf16, tag="scat")
                ab16 = wpool.tile([P, 4, 16], bf16, tag="ab16")
                # a[u] = s1[u] * s2
                nc.vector.tensor_tensor(
                    out=ab16[:, :, 0:8], in0=xs[:, :, 0:8],
                    in1=shw[:, :, 0:1].to_broadcast([P, 4, 8]), op=MUL_,
                )
                # b[u] = sum_i v1[u,i] * v2[i]
                pb = wpool.tile([P, 4, 8, 3], bf16, tag="pb")
                nc.vector.tensor_tensor(
                    out=pb[:],
                    in0=xs[:, :, 8:32].rearrange("p g (u i) -> p g u i", u=8),
                    in1=shw[:, :, 1:4].unsqueeze(2).to_broadcast([P, 4, 8, 3]),
                    op=MUL_,
                )
                with nc.allow_low_precision(reason="3-elem dot, tol 2e-2"):
                    nc.vector.tensor_reduce(
                        out=ab16[:, :, 8:16], in_=pb[:], axis=AX, op=ADD
                    )
                # ps = ab16[u'] * w01[u', w_]  -> scat[:, :, 0:128]
                nc.vector.tensor_tensor(
                    out=scat[:, :, 0:128].rearrange("p g (u w) -> p g u w", u=16),
                    in0=ab16[:].unsqueeze(3).to_broadcast([P, 4, 16, 8]),
                    in1=w_ps[:, :, 0:128].rearrange("p g (u w) -> p g u w", u=16),
                    op=MUL_,
                )
                # z23[t,u,i]: t=0 -> s1[u]*v2[i], t=1 -> v1[u,i]*s2
                z23 = wpool.tile([P, 4, 2, 24], bf16, tag="z23")
                nc.vector.tensor_tensor(
                    out=z23[:, :, 0, :].rearrange("p g (u i) -> p g u i", u=8),
                    in0=xs[:, :, 0:8].unsqueeze(3).to_broadcast([P, 4, 8, 3]),
                    in1=shw[:, :, 1:4].unsqueeze(2).to_broadcast([P, 4, 8, 3]),
                    op=MUL_,
                )
                nc.vector.tensor_tensor(
                    out=z23[:, :, 1, :], in0=xs[:, :, 8:32],
                    in1=shw[:, :, 0:1].to_broadcast([P, 4, 24]), op=MUL_,
                )
                # scat cols 128:512 ((t,u,w,i)) = z23[t,u,i] * w23[t,u,w]
                nc.vector.tensor_tensor(
                    out=scat[:, :, 128:512].rearrange(
                        "p g (t u w i) -> p g t u w i", t=2, u=8, w=8
                    ),
                    in0=z23[:].rearrange("p g t (u i) -> p g t u i", u=8)
                        .unsqueeze(4).to_broadcast([P, 4, 2, 8, 8, 3]),
                    in1=w_ps[:, :, 128:256]
                        .rearrange("p g (t u w) -> p g t u w", t=2, u=8, w=8)
                        .unsqueeze(5).to_broadcast([P, 4, 2, 8, 8, 3]),
                    op=MUL_,
                )

                emit_scatter(g0, oh4, scat)
    nc.compile()
    return nc


# ---------------- host-side prep ----------------
def _prep(node_features, edge_src, edge_dst, edge_sh, edge_scalars, fc_w1, fc_w2,
          schedule):
    GROUPS = sum(schedule)
    EPAD = GROUPS * 128
    Gq = np.zeros(NB + 1, np.int64)
    Gq[1:] = np.cumsum(schedule)
    Gedge = Gq * 128                  # edge-slot start of block slot k
    WINDOWS = GROUPS // 4
    NJ = (WINDOWS + 3) // 4
    # gather call boundaries: 1024-idx supers, possibly a 512 tail
    call_ends = []
    wpos = 0
    while wpos < WINDOWS:
        nidx = 1024 if wpos + 1 < WINDOWS else 512
        call_ends.append(wpos * 512 + nidx)
        wpos += 2

    # fold all scalar coefficients into the weights
    w1s = (fc_w1 * (1.0 / math.sqrt(NUM_BASIS))).astype(np.float32)     # [8, 256]
    w2 = (fc_w2 * (SQRT2 / math.sqrt(HIDDEN))).astype(np.float64)       # [256, 256]
    w2 = w2.reshape(HIDDEN, 4, MUL, MUL)
    coef = np.array(
        [A_SCALAR, A_SCALAR * INV_SQRT3, A_VECTOR * INV_SQRT3, A_VECTOR * INV_SQRT3]
    ) * DEG_SCALE
    w2 = w2 * coef[None, :, None, None]
    # device col order: [w01 (u'16, w8) | w2 (u8, w8) | w3 (u8, w8)]
    w2dev = np.concatenate(
        [
            w2[:, 0].reshape(HIDDEN, 64),
            w2[:, 1].reshape(HIDDEN, 64),
            w2[:, 2].reshape(HIDDEN, 64),
            w2[:, 3].reshape(HIDDEN, 64),
        ],
        axis=1,
    ).astype(np.float32)                                                # [256, 256]

    w1t = np.zeros((P, 256), np.float32)
    for c in range(4):
        w1t[32 * c : 32 * c + 8] = w1s
    w2t = np.zeros((P, 512), np.float32)
    w2t[:, 0:256] = w2dev[0:128]
    w2t[:, 256:512] = w2dev[128:256]
    tbl = np.zeros((TBL_ROWS, TBL_COLS), BF16)
    tbl[1 : N_NODES + 1, 0:32] = node_features

    src_all = np.asarray(edge_src).astype(np.int64)
    dst_all = np.asarray(edge_dst).astype(np.int64)
    es_all = np.asarray(edge_scalars).astype(np.float32)
    sh_all = np.asarray(edge_sh).astype(np.float32)
    core_of = dst_all // NODES_PER_CORE

    in_maps = []
    orders = []
    for cid in range(NCORES):
        sel = np.nonzero(core_of == cid)[0]
        d = dst_all[sel]
        order = np.argsort(d, kind="stable")
        sel = sel[order]
        d = d[order]
        blk = (d - NODES_PER_CORE * cid) >> 7
        cnt = np.bincount(blk, minlength=NB)
        border = np.argsort(-cnt, kind="stable")   # block id for slot k
        orders.append(border)
        kpos = np.empty(NB, np.int64)
        kpos[border] = np.arange(NB)
        assert (cnt[border] <= np.asarray(schedule) * 128).all(), cid
        start = np.zeros(NB, np.int64)
        start[1:] = np.cumsum(cnt)[:-1]
        rank = np.arange(len(sel)) - start[blk]
        slot = Gedge[kpos[blk]] + rank

        srcv = np.full(EPAD, -1, np.int64)
        srcv[slot] = src_all[sel]
        shv = np.zeros((EPAD, 4), np.float32)
        shv[slot] = sh_all[sel]
        esv = np.zeros((EPAD, 8), np.float32)
        esv[slot] = es_all[sel]
        dlv = np.zeros(EPAD, np.float32)
        dlv[slot] = (d - NODES_PER_CORE * cid - 128 * blk).astype(np.float32)

        # --- gather indices: idx = node - (GBASE - 1 - 1)... row = node+1,
        # idx = row - GBASE = node + 1 - GBASE; dummy -> DUMMY_IDX (>= 0)
        idxv = np.where(srcv >= 0, srcv + 1 - GBASE, DUMMY_IDX).astype(np.int64)
        # force the trim-order-last index of each 512-idx gather call to be
        # >= 0 by swapping that edge with a non-negative-idx edge of the SAME
        # node block (any within-block permutation is valid).
        ends = set(call_ends)
        for jl in call_ends:
            jl = jl - 1
            if idxv[jl] >= 0:
                continue
            k0 = int(np.searchsorted(Gedge, jl, side="right")) - 1
            lo, hi = int(Gedge[k0]), int(Gedge[k0 + 1])
            cand = np.nonzero(idxv[lo:hi] >= 0)[0]
            # exclude other calls' final slots
            cand = [lo + q for q in cand if (lo + q + 1) not in ends]
            assert cand, "no swap candidate in block"
            q = cand[0]
            for arr in (idxv, srcv, dlv):
                arr[jl], arr[q] = arr[q], arr[jl]
            for arr in (shv, esv):
                tmpq = arr[q].copy()
                arr[q] = arr[jl]
                arr[jl] = tmpq
        idx_g = np.tile(
            idxv.reshape(-1, 16).T.astype(np.int16), (8, 1)
        )  # wrap is uniform: IDXW*NSUPER cols total

        # es4: window w at rows 32*(w%4)+b, cols [ (w//4)*512, +512 )
        es4 = np.zeros((32, NJ * 512), np.float32)
        esw = esv.reshape(WINDOWS, 512, 8)
        for c in range(4):
            wsel = np.arange(c, WINDOWS, 4)       # these windows use strip c
            nw = len(wsel)                        # w//4 == index within wsel
            es4[8 * c : 8 * c + 8, : nw * 512] = (
                esw[wsel].transpose(2, 0, 1).reshape(8, nw * 512)
            )

        sh_t = shv.reshape(GROUPS, P, 4).transpose(1, 0, 2).reshape(P, GROUPS * 4)
        dl_mat = dlv.reshape(GROUPS, P).astype(np.int32)
        oht = (dl_mat[:, :, None] == np.arange(P, dtype=np.int32)[None, None, :])
        oht = np.ascontiguousarray(
            oht.transpose(1, 0, 2).reshape(P, GROUPS * 128)
        ).astype(mybir.dt.np(mybir.dt.float8e4))

        in_maps.append(
            dict(
                tbl=tbl, idx_g=np.ascontiguousarray(idx_g),
                es4=np.ascontiguousarray(es4).astype(BF16),
                sh_t=np.ascontiguousarray(sh_t).astype(BF16),
                oht=oht,
                w1t=w1t.astype(BF16), w2t=w2t.astype(BF16),
            )
        )
    return in_maps, orders


def _compute_schedule(edge_dst):
    dst_all = np.asarray(edge_dst).astype(np.int64)
    cnt = np.zeros((NCORES, NB), np.int64)
    for c in range(NCORES):
        dl = dst_all[(dst_all // NODES_PER_CORE) == c] - c * NODES_PER_CORE
        cnt[c] = np.bincount(dl >> 7, minlength=NB)
    scnt = -np.sort(-cnt, axis=1)                  # each core's blocks desc
    ng_k = np.maximum(np.ceil(scnt.max(axis=0) / 128.0).astype(np.int64), 1)
    while int(ng_k.sum()) % 4:
        ng_k[-1] += 1
    return tuple(int(x) for x in ng_k)


def kernel(node_features, edge_src, edge_dst, edge_sh, edge_scalars, fc_w1, fc_w2):
    node_features = np.asarray(node_features, dtype=np.float32)
    edge_sh = np.asarray(edge_sh, dtype=np.float32)
    edge_scalars = np.asarray(edge_scalars, dtype=np.float32)
    fc_w1 = np.asarray(fc_w1, dtype=np.float32)
    fc_w2 = np.asarray(fc_w2, dtype=np.float32)

    schedule = _compute_schedule(edge_dst)
    if schedule not in _PROG_CACHE:
        _PROG_CACHE[schedule] = _build_program(schedule)
    nc = _PROG_CACHE[schedule]

    in_maps, orders = _prep(
        node_features, edge_src, edge_dst, edge_sh, edge_scalars, fc_w1, fc_w2,
        schedule,
    )
    res = run_bass_kernel_spmd(nc, in_maps, core_ids=list(range(NCORES)))
    outs = []
    for c in range(NCORES):
        no = res.results[c]["nodeout"].reshape(NB, 128, 32)
        oc = np.empty_like(no)
        oc[orders[c]] = no
        outs.append(oc.reshape(NODES_PER_CORE, 32))
    out = np.concatenate(outs, axis=0)
    return out[:N_NODES].astype(np.float32)
